# revision 11
# baseline (speedup 1.0000x reference)
"""Causal self-attention on 8 Trainium2 NeuronCores.

Sharding: core c handles batch b = c//2 and heads [(c%2)*8, (c%2)*8+8).
Each core computes the full QKV projection for its head slice, causal
flash-style attention, and the row-parallel w_o partial product. The two
partials per batch are summed on the host (no device collectives).

All PE matmuls run in fp16 (1 cycle/row) with fp32 PSUM accumulation.
Feature-major layouts throughout:
  x^T [D, N]        (host pre-transposed)
  Q^T, K^T [ch, N]  (from GEMM with W stationary, x^T moving)
  V [N, ch] + ones  (from GEMM with x^T stationary, W moving)
  S^T [k, q] = K^T_tile.T @ Q^T  -> exp -> P^T [k, q]
  O^T [ch, q] = (V|1).T @ P^T    (row 64 = softmax denominator)
  y = O^T_norm.T @ W_o           (accumulated over ch tiles)

Causal masking (triangular tightening): for a diagonal-straddling S^T
block with offset delta = 128*kt - 512*qc, columns j < delta are fully
masked so S/exp/AV are simply narrowed to cols [delta, 512). The
partially-masked 128-wide sub-block [delta, delta+128) is zeroed after
exp by one DVE multiply with a 0/1 upper-triangular mask.
"""

import numpy as np

B, N, D, H = 4, 2048, 1024, 16
DH = 64
N_CORES = 8
HPC = 8            # heads per core
CH = HPC * DH      # 512 channels per core
SCALE = 1.0 / 8.0  # 1/sqrt(DH)

_cached = None


def _build_program():
    from contextlib import ExitStack

    import concourse.tile as tile
    from concourse import bacc, mybir

    f16 = mybir.dt.float16
    f32 = mybir.dt.float32
    Exp = mybir.ActivationFunctionType.Exp
    mult = mybir.AluOpType.mult
    add = mybir.AluOpType.add

    nc = bacc.Bacc(
        "TRN2", target_bir_lowering=False, debug=False, num_devices=N_CORES
    )

    xT_d = nc.dram_tensor("xT", [D, N], f16, kind="ExternalInput").ap()
    wq_d = nc.dram_tensor("wq", [D, CH], f16, kind="ExternalInput").ap()
    wk_d = nc.dram_tensor("wk", [D, CH], f16, kind="ExternalInput").ap()
    wv_d = nc.dram_tensor("wv", [D, CH], f16, kind="ExternalInput").ap()
    wo_d = nc.dram_tensor("wo", [CH, D], f16, kind="ExternalInput").ap()
    bq_d = nc.dram_tensor("bq", [CH, 1], f32, kind="ExternalInput").ap()
    bk_d = nc.dram_tensor("bk", [CH, 1], f32, kind="ExternalInput").ap()
    bv_d = nc.dram_tensor("bvb", [128, CH], f32, kind="ExternalInput").ap()
    bo_d = nc.dram_tensor("bob", [128, D], f32, kind="ExternalInput").ap()
    msk_d = nc.dram_tensor("msk", [128, 256], f16, kind="ExternalInput").ap()
    y_d = nc.dram_tensor("y", [N, D], f32, kind="ExternalOutput").ap()

    with tile.TileContext(nc) as tc, ExitStack() as ctx:
        const = ctx.enter_context(tc.tile_pool(name="const", bufs=1))
        actp = ctx.enter_context(tc.tile_pool(name="actp", bufs=1))
        work = ctx.enter_context(tc.tile_pool(name="work", bufs=3))
        ps_s = ctx.enter_context(tc.tile_pool(name="ps_s", bufs=2, space="PSUM"))
        ps_av = ctx.enter_context(tc.tile_pool(name="ps_av", bufs=1, space="PSUM"))
        ps_p = ctx.enter_context(tc.tile_pool(name="ps_p", bufs=2, space="PSUM"))

        # ---- constants / weights into SBUF ----
        # K-weights + first seq-chunk of x first so the K^T GEMM starts ASAP.
        wq = [const.tile([128, CH], f16, tag=f"wq{i}", name=f"wq{i}") for i in range(8)]
        wk = [const.tile([128, CH], f16, tag=f"wk{i}", name=f"wk{i}") for i in range(8)]
        wv = [const.tile([128, CH], f16, tag=f"wv{i}", name=f"wv{i}") for i in range(8)]
        xt = [[const.tile([128, 512], f16, tag=f"xt{i}_{sc}", name=f"xt{i}_{sc}")
               for sc in range(4)] for i in range(8)]
        # Round-robin input DMAs across engine queues so the 2D
        # row-descriptor processing runs in parallel; first-needed first.
        engs = [nc.sync, nc.gpsimd]
        _ei = [0]

        def dma_in(dst, src):
            engs[_ei[0] % len(engs)].dma_start(dst, src)
            _ei[0] += 1

        for i in range(8):
            dma_in(wk[i][:], wk_d[i * 128 : (i + 1) * 128, :])
            dma_in(xt[i][0][:], xT_d[i * 128 : (i + 1) * 128, 0:512])
        bq = [const.tile([128, 1], f32, tag=f"bq{j}", name=f"bq{j}") for j in range(4)]
        bk = [const.tile([128, 1], f32, tag=f"bk{j}", name=f"bk{j}") for j in range(4)]
        for j in range(4):
            dma_in(bq[j][:], bq_d[j * 128 : (j + 1) * 128, :])
            dma_in(bk[j][:], bk_d[j * 128 : (j + 1) * 128, :])
        bv_t = const.tile([128, CH], f32, tag="bvb", name="bvb")
        dma_in(bv_t[:], bv_d[:])
        for i in range(8):
            dma_in(wv[i][:], wv_d[i * 128 : (i + 1) * 128, :])
            dma_in(wq[i][:], wq_d[i * 128 : (i + 1) * 128, :])
        for sc in range(1, 4):
            for i in range(8):
                dma_in(xt[i][sc][:],
                       xT_d[i * 128 : (i + 1) * 128, sc * 512 : (sc + 1) * 512])
        msk_t = const.tile([128, 256], f16, tag="msk", name="msk")
        dma_in(msk_t[:], msk_d[:])
        wo = [const.tile([128, D], f16, tag=f"wo{j}", name=f"wo{j}") for j in range(4)]
        for j in range(4):
            dma_in(wo[j][:], wo_d[j * 128 : (j + 1) * 128, :])
        bo_t = const.tile([128, D], f32, tag="bob", name="bob")
        dma_in(bo_t[:], bo_d[:])

        # ---- persistent activations ----
        QT = [[actp.tile([128, 512], f16, tag=f"qt{ct}_{sc}", name=f"qt{ct}_{sc}") for sc in range(4)]
              for ct in range(4)]
        KT = [[actp.tile([128, 512], f16, tag=f"kt{ct}_{sc}", name=f"kt{ct}_{sc}") for sc in range(4)]
              for ct in range(4)]
        V = [actp.tile([128, 8 * 65], f16, tag=f"v{st}", name=f"v{st}") for st in range(16)]
        OTn = [[actp.tile([128, 512], f16, tag=f"otn{hp}_{qc}", name=f"otn{hp}_{qc}") for qc in range(4)]
               for hp in range(4)]

        # ---- fused pipeline: per seq-chunk sc emit QKV(sc), then the
        # PREVIOUS chunk's out-projection (fills the PE while this chunk's
        # softmax normalization chain drains), then attention qc=sc.
        def emit_outproj(qc, stls=(0, 1, 2, 3)):
            for stl in stls:
                st = 4 * qc + stl
                sl = slice(stl * 128, (stl + 1) * 128)
                for oc in range(2):
                    ocs = slice(oc * 512, (oc + 1) * 512)
                    yp = ps_p.tile([128, 512], f32, tag="p512", name="p512")
                    for hpp in range(4):
                        nc.tensor.matmul(yp[:], OTn[hpp][qc][:, sl],
                                         wo[hpp][:, ocs],
                                         start=(hpp == 0), stop=(hpp == 3),
                                         skip_group_check=True)
                    ysb = work.tile([128, 512], f32, tag="ysb", name="ysb")
                    nc.vector.scalar_tensor_tensor(ysb[:], yp[:], 1.0,
                                                   bo_t[:, ocs], mult, add)
                    nc.sync.dma_start(y_d[st * 128 : (st + 1) * 128, ocs],
                                      ysb[:])

        for sc in range(4):
            # QKV projections for this seq-chunk. Q first: attention(qc>=1)
            # is gated on this chunk's Q (older K/V chunks are already done).
            for ct in range(4):
                cs = slice(ct * 128, (ct + 1) * 128)
                p = ps_p.tile([128, 512], f32, tag="p512", name="p512")
                for d in range(8):
                    nc.tensor.matmul(p[:], wq[d][:, cs], xt[d][sc][:],
                                     start=(d == 0), stop=(d == 7))
                nc.vector.tensor_scalar_add(QT[ct][sc][:], p[:], bq[ct][:])
            for ct in range(4):
                cs = slice(ct * 128, (ct + 1) * 128)
                p = ps_p.tile([128, 512], f32, tag="p512", name="p512")
                for d in range(8):
                    nc.tensor.matmul(p[:], wk[d][:, cs], xt[d][sc][:],
                                     start=(d == 0), stop=(d == 7))
                nc.vector.tensor_scalar_add(KT[ct][sc][:], p[:], bk[ct][:])
            for stl in range(4):
                st = 4 * sc + stl
                ts = slice(stl * 128, (stl + 1) * 128)
                p = ps_p.tile([128, 512], f32, tag="p512", name="p512")
                for d in range(8):
                    nc.tensor.matmul(p[:], xt[d][sc][:, ts], wv[d][:, :],
                                     start=(d == 0), stop=(d == 7))
                v3 = V[st][:].rearrange("p (h e) -> p h e", e=65)
                nc.vector.scalar_tensor_tensor(
                    v3[:, :, 0:64],
                    p[:].rearrange("p (h e) -> p h e", e=64),
                    1.0,
                    bv_t[:].rearrange("p (h e) -> p h e", e=64),
                    mult, add,
                )
                nc.vector.memset(v3[:, :, 64:65], 1.0)

            # attention for query chunk qc = sc, head pairs interleaved so the
            # even head's K=64 matmuls (rows 0-63) and the odd head's (rows
            # 64-127) run concurrently in the PE array.
            qc = sc
            nkt = 4 * (qc + 1)
            for hp in range(4):
                h0, h1 = 2 * hp, 2 * hp + 1
                # one 2-bank tile: cols 0-511 head h0, 512-1023 head h1
                av = ps_av.tile([65, 1024], f32, tag="av", name="av")

                # software pipeline: S(kt) -> exp(kt) on ACT while PE runs
                # S(kt+1); AV(kt) issues after S(kt+1) so PE never waits exp.
                pend = []  # (kt, pt tile) awaiting AV

                def emit_av(kt, pt, first, last):
                    delta = max(0, 128 * kt - 512 * qc)
                    nc.tensor.matmul(
                        av[:, delta:512], V[kt][:, h0 * 65 : h0 * 65 + 65],
                        pt[:, delta:512],
                        start=first, stop=last, skip_group_check=True)
                    nc.tensor.matmul(
                        av[:, 512 + delta : 1024], V[kt][:, h1 * 65 : h1 * 65 + 65],
                        pt[:, 512 + delta : 1024],
                        start=first, stop=last, skip_group_check=True)

                for kt in range(nkt):
                    # merged S^T tile: cols 0-511 head h0, 512-1023 head h1
                    sp = ps_s.tile([128, 1024], f32, tag="s2", name="sp")
                    kcol = slice((kt % 4) * 128, (kt % 4) * 128 + 128)
                    diag = kt >= 4 * qc
                    delta = max(0, 128 * kt - 512 * qc)
                    nc.tensor.matmul(
                        sp[:, delta:512], KT[hp][kt // 4][0:64, kcol],
                        QT[hp][qc][0:64, delta:512],
                        start=True, stop=True, skip_group_check=True)
                    nc.tensor.matmul(
                        sp[:, 512 + delta : 1024], KT[hp][kt // 4][64:128, kcol],
                        QT[hp][qc][64:128, delta:512],
                        start=True, stop=True, skip_group_check=True)
                    pt = work.tile([128, 1024], f16, tag="pt", name="pt")
                    sp3 = sp[:].rearrange("p (h e) -> p h e", e=512)
                    pt3 = pt[:].rearrange("p (h e) -> p h e", e=512)
                    nc.scalar.activation(pt3[:, :, delta:512],
                                         sp3[:, :, delta:512], Exp, scale=SCALE)
                    if diag:
                        # zero the partially-masked 128-wide sub-block via a
                        # 0/1 upper-triangular mask (both heads in one op)
                        nc.vector.tensor_mul(
                            pt3[:, :, delta : delta + 128],
                            pt3[:, :, delta : delta + 128],
                            msk_t[:].rearrange("p (h e) -> p h e", e=128),
                        )
                    pend.append((kt, pt))
                    if len(pend) > 1:
                        k0, p0 = pend.pop(0)
                        emit_av(k0, p0, k0 == 0, False)
                k0, p0 = pend.pop(0)
                emit_av(k0, p0, k0 == 0, True)
                # softmax normalization per head (row 64 of av = denom).
                # reciprocal_approx_fast misreads PSUM sources, so stage the
                # denominator row into SBUF via ACT (it idles here), then one
                # fast reciprocal covering both heads.
                den = work.tile([1, 1024], f32, tag="den", name="den")
                nc.scalar.copy(den[:], av[64:65, :])
                rr = work.tile([1, 1024], f32, tag="r", name="rr")
                nc.vector.reciprocal_approx_fast(rr[:], den[:])
                rb0 = work.tile([64, 512], f32, tag="rb", name="rb0")
                nc.gpsimd.partition_broadcast(rb0[:], rr[0:1, 0:512], channels=64)
                nc.vector.tensor_mul(OTn[hp][qc][0:64, :], av[0:64, 0:512], rb0[:])
                rb1 = work.tile([64, 512], f32, tag="rb", name="rb1")
                nc.gpsimd.partition_broadcast(rb1[:], rr[0:1, 512:1024], channels=64)
                nc.vector.tensor_mul(OTn[hp][qc][64:128, :], av[0:64, 512:1024],
                                     rb1[:])

                # out-projections for chunks 0-2 are deferred into the
                # (ACT-bound) attention window of chunk 3, where the PE
                # otherwise starves; hold back two chunk-2 groups to fill
                # the final normalization-chain stall.
                if sc == 3 and hp < 3:
                    emit_outproj(hp, (0, 1, 2, 3) if hp < 2 else (0, 1, 2))
        emit_outproj(2, (3,))
        emit_outproj(3)

    nc.compile()
    return nc


def _host_inputs(x, w_qkv, b_qkv, w_o, b_o):
    """Per-core input dicts implementing the sharding + layout prep."""
    # 0/1 upper-triangular mask (keep col >= row), duplicated for both heads
    M = np.triu(np.ones((128, 128), np.float16))
    msk = np.concatenate([M, M], axis=1)

    in_maps = []
    for c in range(N_CORES):
        b = c // 2
        hs = (c % 2) * HPC
        cols = slice(hs * DH, (hs + HPC) * DH)
        in_maps.append({
            "xT": np.ascontiguousarray(x[b].T).astype(np.float16),
            "wq": w_qkv[:, cols].astype(np.float16),
            "wk": w_qkv[:, D:][:, cols].astype(np.float16),
            "wv": w_qkv[:, 2 * D:][:, cols].astype(np.float16),
            "wo": w_o[hs * DH : (hs + HPC) * DH, :].astype(np.float16),
            "bq": b_qkv[cols].reshape(CH, 1).astype(np.float32),
            "bk": b_qkv[D:][cols].reshape(CH, 1).astype(np.float32),
            "bvb": np.tile(b_qkv[2 * D:][cols].astype(np.float32), (128, 1)),
            "bob": np.tile(b_o.astype(np.float32), (128, 1)),
            "msk": msk,
        })
    return in_maps


def kernel(x, w_qkv, b_qkv, w_o, b_o):
    global _cached
    from concourse.bass_utils import run_bass_kernel_spmd

    x = np.asarray(x)
    w_qkv = np.asarray(w_qkv)
    b_qkv = np.asarray(b_qkv)
    w_o = np.asarray(w_o)
    b_o = np.asarray(b_o)

    if _cached is None:
        _cached = _build_program()
    nc = _cached

    in_maps = _host_inputs(x, w_qkv, b_qkv, w_o, b_o)
    res = run_bass_kernel_spmd(nc, in_maps, list(range(N_CORES)))

    out = np.empty((B, N, D), np.float32)
    for b in range(B):
        out[b] = res.results[2 * b]["y"] + res.results[2 * b + 1]["y"]
    return out


# revision 15
# speedup vs baseline: 1.1808x; 1.1808x over previous
"""Causal self-attention on 8 Trainium2 NeuronCores.

Sharding: core c handles batch b = c//2 and heads [(c%2)*8, (c%2)*8+8).
Each core computes the full QKV projection for its head slice, causal
flash-style attention, and the row-parallel w_o partial product. The two
partials per batch are summed on the host (no device collectives).

All PE matmuls run in fp16 (1 cycle/row) with fp32 PSUM accumulation.
Feature-major layouts throughout:
  x^T [D, N]        (host pre-transposed)
  Q^T, K^T [ch, N]  (from GEMM with W stationary, x^T moving)
  V [N, ch] + ones  (from GEMM with x^T stationary, W moving)
  S^T [k, q] = K^T_tile.T @ Q^T  -> exp -> P^T [k, q]
  O^T [ch, q] = (V|1).T @ P^T    (row 64 = softmax denominator)
  y = O^T_norm.T @ W_o           (accumulated over ch tiles)

Causal masking (triangular tightening): for a diagonal-straddling S^T
block with offset delta = 128*kt - 512*qc, columns j < delta are fully
masked so S/exp/AV are simply narrowed to cols [delta, 512). The
partially-masked 128-wide sub-block [delta, delta+128) is zeroed after
exp by one DVE multiply with a 0/1 upper-triangular mask.
"""

import numpy as np

B, N, D, H = 4, 2048, 1024, 16
DH = 64
N_CORES = 8
HPC = 8            # heads per core
CH = HPC * DH      # 512 channels per core
SCALE = 1.0 / 8.0  # 1/sqrt(DH)

_cached = None


def _build_program():
    from contextlib import ExitStack

    import concourse.tile as tile
    from concourse import bacc, mybir

    f16 = mybir.dt.float16
    f32 = mybir.dt.float32
    Exp = mybir.ActivationFunctionType.Exp
    mult = mybir.AluOpType.mult
    add = mybir.AluOpType.add

    nc = bacc.Bacc(
        "TRN2", target_bir_lowering=False, debug=False, num_devices=N_CORES
    )

    xT_d = nc.dram_tensor("xT", [D, N], f16, kind="ExternalInput").ap()
    wq_d = nc.dram_tensor("wq", [D, CH], f16, kind="ExternalInput").ap()
    wk_d = nc.dram_tensor("wk", [D, CH], f16, kind="ExternalInput").ap()
    wv_d = nc.dram_tensor("wv", [D, CH], f16, kind="ExternalInput").ap()
    wo_d = nc.dram_tensor("wo", [CH, D], f16, kind="ExternalInput").ap()
    bq_d = nc.dram_tensor("bq", [CH, 1], f32, kind="ExternalInput").ap()
    bk_d = nc.dram_tensor("bk", [CH, 1], f32, kind="ExternalInput").ap()
    bv_d = nc.dram_tensor("bvb", [128, CH], f32, kind="ExternalInput").ap()
    bo_d = nc.dram_tensor("bob", [128, D], f32, kind="ExternalInput").ap()
    msk_d = nc.dram_tensor("msk", [128, 256], f16, kind="ExternalInput").ap()
    y_d = nc.dram_tensor("y", [N, D], f32, kind="ExternalOutput").ap()

    with tile.TileContext(nc) as tc, ExitStack() as ctx:
        const = ctx.enter_context(tc.tile_pool(name="const", bufs=1))
        actp = ctx.enter_context(tc.tile_pool(name="actp", bufs=1))
        work = ctx.enter_context(tc.tile_pool(name="work", bufs=3))
        ptp = ctx.enter_context(tc.tile_pool(name="ptp", bufs=5))
        ps_s = ctx.enter_context(tc.tile_pool(name="ps_s", bufs=2, space="PSUM"))
        ps_av = ctx.enter_context(tc.tile_pool(name="ps_av", bufs=1, space="PSUM"))
        ps_p = ctx.enter_context(tc.tile_pool(name="ps_p", bufs=2, space="PSUM"))

        # ---- constants / weights into SBUF ----
        # K-weights + first seq-chunk of x first so the K^T GEMM starts ASAP.
        wq = [const.tile([128, CH], f16, tag=f"wq{i}", name=f"wq{i}") for i in range(8)]
        wk = [const.tile([128, CH], f16, tag=f"wk{i}", name=f"wk{i}") for i in range(8)]
        wv = [const.tile([128, CH], f16, tag=f"wv{i}", name=f"wv{i}") for i in range(8)]
        xt = [[const.tile([128, 512], f16, tag=f"xt{i}_{sc}", name=f"xt{i}_{sc}")
               for sc in range(4)] for i in range(8)]
        # Round-robin input DMAs across engine queues so the 2D
        # row-descriptor processing runs in parallel; first-needed first.
        engs = [nc.sync, nc.gpsimd]
        _ei = [0]

        def dma_in(dst, src):
            engs[_ei[0] % len(engs)].dma_start(dst, src)
            _ei[0] += 1

        for i in range(8):
            dma_in(wq[i][:], wq_d[i * 128 : (i + 1) * 128, :])
            dma_in(xt[i][0][:], xT_d[i * 128 : (i + 1) * 128, 0:512])
        bq = [const.tile([128, 1], f32, tag=f"bq{j}", name=f"bq{j}") for j in range(4)]
        bk = [const.tile([128, 1], f32, tag=f"bk{j}", name=f"bk{j}") for j in range(4)]
        for j in range(4):
            dma_in(bq[j][:], bq_d[j * 128 : (j + 1) * 128, :])
            dma_in(bk[j][:], bk_d[j * 128 : (j + 1) * 128, :])
        for i in range(8):
            dma_in(wk[i][:], wk_d[i * 128 : (i + 1) * 128, :])
        bv_t = const.tile([128, CH], f32, tag="bvb", name="bvb")
        dma_in(bv_t[:], bv_d[:])
        msk_t = const.tile([128, 256], f16, tag="msk", name="msk")
        dma_in(msk_t[:], msk_d[:])
        for i in range(8):
            dma_in(wv[i][:], wv_d[i * 128 : (i + 1) * 128, :])
        for sc in range(1, 4):
            for i in range(8):
                dma_in(xt[i][sc][:],
                       xT_d[i * 128 : (i + 1) * 128, sc * 512 : (sc + 1) * 512])
        wo = [const.tile([128, D], f16, tag=f"wo{j}", name=f"wo{j}") for j in range(4)]
        for j in range(4):
            dma_in(wo[j][:], wo_d[j * 128 : (j + 1) * 128, :])
        bo_t = const.tile([128, D], f32, tag="bob", name="bob")
        dma_in(bo_t[:], bo_d[:])

        # ---- persistent activations ----
        QT = [[actp.tile([128, 512], f16, tag=f"qt{ct}_{sc}", name=f"qt{ct}_{sc}") for sc in range(4)]
              for ct in range(4)]
        KT = [[actp.tile([128, 512], f16, tag=f"kt{ct}_{sc}", name=f"kt{ct}_{sc}") for sc in range(4)]
              for ct in range(4)]
        V = [actp.tile([128, 8 * 65], f16, tag=f"v{st}", name=f"v{st}") for st in range(16)]
        OTn = [[actp.tile([128, 512], f16, tag=f"otn{hp}_{qc}", name=f"otn{hp}_{qc}") for qc in range(4)]
               for hp in range(4)]

        # ---- fused pipeline: per seq-chunk sc emit QKV(sc), then the
        # PREVIOUS chunk's out-projection (fills the PE while this chunk's
        # softmax normalization chain drains), then attention qc=sc.
        def emit_outproj(qc, stls=(0, 1, 2, 3)):
            for stl in stls:
                st = 4 * qc + stl
                sl = slice(stl * 128, (stl + 1) * 128)
                for oc in range(2):
                    ocs = slice(oc * 512, (oc + 1) * 512)
                    yp = ps_p.tile([128, 512], f32, tag="p512", name="p512")
                    for hpp in range(4):
                        nc.tensor.matmul(yp[:], OTn[hpp][qc][:, sl],
                                         wo[hpp][:, ocs],
                                         start=(hpp == 0), stop=(hpp == 3),
                                         skip_group_check=True)
                    ysb = work.tile([128, 512], f32, tag="ysb", name="ysb")
                    nc.vector.scalar_tensor_tensor(ysb[:], yp[:], 1.0,
                                                   bo_t[:, ocs], mult, add)
                    nc.sync.dma_start(y_d[st * 128 : (st + 1) * 128, ocs],
                                      ysb[:])

        for sc in range(4):
            # QKV projections for this seq-chunk. Q first: attention(qc>=1)
            # is gated on this chunk's Q (older K/V chunks are already done).
            for ct in range(4):
                cs = slice(ct * 128, (ct + 1) * 128)
                p = ps_p.tile([128, 512], f32, tag="p512", name="p512")
                for d in range(8):
                    nc.tensor.matmul(p[:], wq[d][:, cs], xt[d][sc][:],
                                     start=(d == 0), stop=(d == 7))
                nc.vector.tensor_scalar_add(QT[ct][sc][:], p[:], bq[ct][:])
            for ct in range(4):
                cs = slice(ct * 128, (ct + 1) * 128)
                p = ps_p.tile([128, 512], f32, tag="p512", name="p512")
                for d in range(8):
                    nc.tensor.matmul(p[:], wk[d][:, cs], xt[d][sc][:],
                                     start=(d == 0), stop=(d == 7))
                nc.vector.tensor_scalar_add(KT[ct][sc][:], p[:], bk[ct][:])
            for stl in range(4):
                st = 4 * sc + stl
                ts = slice(stl * 128, (stl + 1) * 128)
                p = ps_p.tile([128, 512], f32, tag="p512", name="p512")
                for d in range(8):
                    nc.tensor.matmul(p[:], xt[d][sc][:, ts], wv[d][:, :],
                                     start=(d == 0), stop=(d == 7))
                v3 = V[st][:].rearrange("p (h e) -> p h e", e=65)
                nc.vector.scalar_tensor_tensor(
                    v3[:, :, 0:64],
                    p[:].rearrange("p (h e) -> p h e", e=64),
                    1.0,
                    bv_t[:].rearrange("p (h e) -> p h e", e=64),
                    mult, add,
                )
                nc.vector.memset(v3[:, :, 64:65], 1.0)

            # attention for query chunk qc = sc, head pairs interleaved so the
            # even head's K=64 matmuls (rows 0-63) and the odd head's (rows
            # 64-127) run concurrently in the PE array.
            qc = sc
            nkt = 4 * (qc + 1)
            for hp in range(4):
                h0, h1 = 2 * hp, 2 * hp + 1
                # one 2-bank tile: cols 0-511 head h0, 512-1023 head h1
                av = ps_av.tile([65, 1024], f32, tag="av", name="av")

                # software pipeline: S(kt) -> exp(kt) on ACT while PE runs
                # S(kt+1); AV(kt) issues after S(kt+1) so PE never waits exp.
                pend = []  # (kt, pt tile) awaiting AV

                def emit_av(kt, pt, first, last):
                    delta = max(0, 128 * kt - 512 * qc)
                    nc.tensor.matmul(
                        av[:, delta:512], V[kt][:, h0 * 65 : h0 * 65 + 65],
                        pt[:, delta:512],
                        start=first, stop=last, skip_group_check=True)
                    nc.tensor.matmul(
                        av[:, 512 + delta : 1024], V[kt][:, h1 * 65 : h1 * 65 + 65],
                        pt[:, 512 + delta : 1024],
                        start=first, stop=last, skip_group_check=True)

                for kt in range(nkt):
                    # merged S^T tile: cols 0-511 head h0, 512-1023 head h1
                    sp = ps_s.tile([128, 1024], f32, tag="s2", name="sp")
                    kcol = slice((kt % 4) * 128, (kt % 4) * 128 + 128)
                    diag = kt >= 4 * qc
                    delta = max(0, 128 * kt - 512 * qc)
                    nc.tensor.matmul(
                        sp[:, delta:512], KT[hp][kt // 4][0:64, kcol],
                        QT[hp][qc][0:64, delta:512],
                        start=True, stop=True, skip_group_check=True)
                    nc.tensor.matmul(
                        sp[:, 512 + delta : 1024], KT[hp][kt // 4][64:128, kcol],
                        QT[hp][qc][64:128, delta:512],
                        start=True, stop=True, skip_group_check=True)
                    pt = ptp.tile([128, 1024], f16, tag="pt", name="pt")
                    sp3 = sp[:].rearrange("p (h e) -> p h e", e=512)
                    pt3 = pt[:].rearrange("p (h e) -> p h e", e=512)
                    nc.scalar.activation(pt3[:, :, delta:512],
                                         sp3[:, :, delta:512], Exp, scale=SCALE)
                    if diag:
                        # zero the partially-masked 128-wide sub-block via a
                        # 0/1 upper-triangular mask (both heads in one op)
                        nc.vector.tensor_mul(
                            pt3[:, :, delta : delta + 128],
                            pt3[:, :, delta : delta + 128],
                            msk_t[:].rearrange("p (h e) -> p h e", e=128),
                        )
                    pend.append((kt, pt))
                    if len(pend) > 1:
                        k0, p0 = pend.pop(0)
                        emit_av(k0, p0, k0 == 0, False)
                k0, p0 = pend.pop(0)
                emit_av(k0, p0, k0 == 0, True)
                # softmax normalization per head (row 64 of av = denom).
                # reciprocal_approx_fast misreads PSUM sources, so stage the
                # denominator row into SBUF first (on DVE: the strict ACT
                # FIFO would head-of-line-block the next head-pair's exps).
                den = work.tile([1, 1024], f32, tag="den", name="den")
                nc.vector.tensor_copy(den[:], av[64:65, :])
                rr = work.tile([1, 1024], f32, tag="r", name="rr")
                nc.vector.reciprocal_approx_fast(rr[:], den[:])
                rb0 = work.tile([64, 512], f32, tag="rb", name="rb0")
                nc.gpsimd.partition_broadcast(rb0[:], rr[0:1, 0:512], channels=64)
                nc.vector.tensor_mul(OTn[hp][qc][0:64, :], av[0:64, 0:512], rb0[:])
                rb1 = work.tile([64, 512], f32, tag="rb", name="rb1")
                nc.gpsimd.partition_broadcast(rb1[:], rr[0:1, 512:1024], channels=64)
                nc.vector.tensor_mul(OTn[hp][qc][64:128, :], av[0:64, 512:1024],
                                     rb1[:])

                # out-projections for chunks 0-2 are deferred into the
                # (ACT-bound) attention window of chunk 3, where the PE
                # otherwise starves; hold back two chunk-2 groups to fill
                # the final normalization-chain stall.
                if sc == 3 and hp < 3:
                    emit_outproj(hp, (0, 1, 2, 3) if hp < 2 else (0, 1, 2))
        emit_outproj(2, (3,))
        emit_outproj(3)

    nc.compile()
    return nc


def _host_inputs(x, w_qkv, b_qkv, w_o, b_o):
    """Per-core input dicts implementing the sharding + layout prep."""
    # 0/1 upper-triangular mask (keep col >= row), duplicated for both heads
    M = np.triu(np.ones((128, 128), np.float16))
    msk = np.concatenate([M, M], axis=1)

    in_maps = []
    for c in range(N_CORES):
        b = c // 2
        hs = (c % 2) * HPC
        cols = slice(hs * DH, (hs + HPC) * DH)
        in_maps.append({
            "xT": np.ascontiguousarray(x[b].T).astype(np.float16),
            "wq": w_qkv[:, cols].astype(np.float16),
            "wk": w_qkv[:, D:][:, cols].astype(np.float16),
            "wv": w_qkv[:, 2 * D:][:, cols].astype(np.float16),
            "wo": w_o[hs * DH : (hs + HPC) * DH, :].astype(np.float16),
            "bq": b_qkv[cols].reshape(CH, 1).astype(np.float32),
            "bk": b_qkv[D:][cols].reshape(CH, 1).astype(np.float32),
            "bvb": np.tile(b_qkv[2 * D:][cols].astype(np.float32), (128, 1)),
            "bob": np.tile(b_o.astype(np.float32), (128, 1)),
            "msk": msk,
        })
    return in_maps


def kernel(x, w_qkv, b_qkv, w_o, b_o):
    global _cached
    from concourse.bass_utils import run_bass_kernel_spmd

    x = np.asarray(x)
    w_qkv = np.asarray(w_qkv)
    b_qkv = np.asarray(b_qkv)
    w_o = np.asarray(w_o)
    b_o = np.asarray(b_o)

    if _cached is None:
        _cached = _build_program()
    nc = _cached

    in_maps = _host_inputs(x, w_qkv, b_qkv, w_o, b_o)
    res = run_bass_kernel_spmd(nc, in_maps, list(range(N_CORES)))

    out = np.empty((B, N, D), np.float32)
    for b in range(B):
        out[b] = res.results[2 * b]["y"] + res.results[2 * b + 1]["y"]
    return out


# revision 18
# speedup vs baseline: 1.2565x; 1.0641x over previous
"""Causal self-attention on 8 Trainium2 NeuronCores.

Sharding: core c handles batch b = c//2 and heads [(c%2)*8, (c%2)*8+8).
Each core computes the full QKV projection for its head slice, causal
flash-style attention, and the row-parallel w_o partial product. The two
partials per batch are summed on the host (no device collectives).

All PE matmuls run in fp16 (1 cycle/row) with fp32 PSUM accumulation.
Feature-major layouts throughout:
  x^T [D, N]        (host pre-transposed)
  Q^T, K^T [ch, N]  (from GEMM with W stationary, x^T moving)
  V [N, ch] + ones  (from GEMM with x^T stationary, W moving)
  S^T [k, q] = K^T_tile.T @ Q^T  -> exp -> P^T [k, q]
  O^T [ch, q] = (V|1).T @ P^T    (row 64 = softmax denominator)
  y = O^T_norm.T @ W_o           (accumulated over ch tiles)

Causal masking (triangular tightening): for a diagonal-straddling S^T
block with offset delta = 128*kt - 512*qc, columns j < delta are fully
masked so S/exp/AV are simply narrowed to cols [delta, 512). The
partially-masked 128-wide sub-block [delta, delta+128) is zeroed after
exp by one DVE multiply with a 0/1 upper-triangular mask.
"""

import numpy as np

B, N, D, H = 4, 2048, 1024, 16
DH = 64
N_CORES = 8
HPC = 8            # heads per core
CH = HPC * DH      # 512 channels per core
SCALE = 1.0 / 8.0  # 1/sqrt(DH)

_cached = None


def _build_program():
    from contextlib import ExitStack

    import concourse.tile as tile
    from concourse import bacc, mybir

    f16 = mybir.dt.float16
    f32 = mybir.dt.float32
    Exp = mybir.ActivationFunctionType.Exp
    mult = mybir.AluOpType.mult
    add = mybir.AluOpType.add

    nc = bacc.Bacc(
        "TRN2", target_bir_lowering=False, debug=False, num_devices=N_CORES
    )

    xT_d = nc.dram_tensor("xT", [D, N], f16, kind="ExternalInput").ap()
    wq_d = nc.dram_tensor("wq", [D, CH], f16, kind="ExternalInput").ap()
    wk_d = nc.dram_tensor("wk", [D, CH], f16, kind="ExternalInput").ap()
    wv_d = nc.dram_tensor("wv", [D, CH], f16, kind="ExternalInput").ap()
    wo_d = nc.dram_tensor("wo", [CH, D], f16, kind="ExternalInput").ap()
    bq_d = nc.dram_tensor("bq", [CH, 1], f32, kind="ExternalInput").ap()
    bk_d = nc.dram_tensor("bk", [CH, 1], f32, kind="ExternalInput").ap()
    bv_d = nc.dram_tensor("bvb", [128, CH], f32, kind="ExternalInput").ap()
    bo_d = nc.dram_tensor("bob", [128, D], f32, kind="ExternalInput").ap()
    msk_d = nc.dram_tensor("msk", [128, 256], f16, kind="ExternalInput").ap()
    y_d = nc.dram_tensor("y", [N, D], f32, kind="ExternalOutput").ap()

    with tile.TileContext(nc) as tc, ExitStack() as ctx:
        const = ctx.enter_context(tc.tile_pool(name="const", bufs=1))
        actp = ctx.enter_context(tc.tile_pool(name="actp", bufs=1))
        work = ctx.enter_context(tc.tile_pool(name="work", bufs=3))
        ptp = ctx.enter_context(tc.tile_pool(name="ptp", bufs=5))
        ps_s = ctx.enter_context(tc.tile_pool(name="ps_s", bufs=2, space="PSUM"))
        ps_av = ctx.enter_context(tc.tile_pool(name="ps_av", bufs=2, space="PSUM"))
        ps_p = ctx.enter_context(tc.tile_pool(name="ps_p", bufs=2, space="PSUM"))

        # ---- constants / weights into SBUF ----
        # K-weights + first seq-chunk of x first so the K^T GEMM starts ASAP.
        wq = [const.tile([128, CH], f16, tag=f"wq{i}", name=f"wq{i}") for i in range(8)]
        wk = [const.tile([128, CH], f16, tag=f"wk{i}", name=f"wk{i}") for i in range(8)]
        wv = [const.tile([128, CH], f16, tag=f"wv{i}", name=f"wv{i}") for i in range(8)]
        xt = [[const.tile([128, 512], f16, tag=f"xt{i}_{sc}", name=f"xt{i}_{sc}")
               for sc in range(4)] for i in range(8)]
        # Round-robin input DMAs across engine queues so the 2D
        # row-descriptor processing runs in parallel; first-needed first.
        engs = [nc.sync, nc.gpsimd]
        _ei = [0]

        def dma_in(dst, src):
            engs[_ei[0] % len(engs)].dma_start(dst, src)
            _ei[0] += 1

        for i in range(8):
            dma_in(wq[i][:], wq_d[i * 128 : (i + 1) * 128, :])
            dma_in(xt[i][0][:], xT_d[i * 128 : (i + 1) * 128, 0:512])
        bq = [const.tile([128, 1], f32, tag=f"bq{j}", name=f"bq{j}") for j in range(4)]
        bk = [const.tile([128, 1], f32, tag=f"bk{j}", name=f"bk{j}") for j in range(4)]
        for j in range(4):
            dma_in(bq[j][:], bq_d[j * 128 : (j + 1) * 128, :])
            dma_in(bk[j][:], bk_d[j * 128 : (j + 1) * 128, :])
        for i in range(8):
            dma_in(wk[i][:], wk_d[i * 128 : (i + 1) * 128, :])
        bv_t = const.tile([128, CH], f32, tag="bvb", name="bvb")
        dma_in(bv_t[:], bv_d[:])
        msk_t = const.tile([128, 256], f16, tag="msk", name="msk")
        dma_in(msk_t[:], msk_d[:])
        for i in range(8):
            dma_in(wv[i][:], wv_d[i * 128 : (i + 1) * 128, :])
        for sc in range(1, 4):
            for i in range(8):
                dma_in(xt[i][sc][:],
                       xT_d[i * 128 : (i + 1) * 128, sc * 512 : (sc + 1) * 512])
        wo = [const.tile([128, D], f16, tag=f"wo{j}", name=f"wo{j}") for j in range(4)]
        for j in range(4):
            dma_in(wo[j][:], wo_d[j * 128 : (j + 1) * 128, :])
        bo_t = const.tile([128, D], f32, tag="bob", name="bob")
        dma_in(bo_t[:], bo_d[:])

        # ---- persistent activations ----
        QT = [[actp.tile([128, 512], f16, tag=f"qt{ct}_{sc}", name=f"qt{ct}_{sc}") for sc in range(4)]
              for ct in range(4)]
        KT = [[actp.tile([128, 512], f16, tag=f"kt{ct}_{sc}", name=f"kt{ct}_{sc}") for sc in range(4)]
              for ct in range(4)]
        V = [actp.tile([128, 8 * 65], f16, tag=f"v{st}", name=f"v{st}") for st in range(16)]
        OTn = [[actp.tile([128, 512], f16, tag=f"otn{hp}_{qc}", name=f"otn{hp}_{qc}") for qc in range(4)]
               for hp in range(4)]

        # ---- fused pipeline: per seq-chunk sc emit QKV(sc), then the
        # PREVIOUS chunk's out-projection (fills the PE while this chunk's
        # softmax normalization chain drains), then attention qc=sc.
        def emit_outproj(qc, stls=(0, 1, 2, 3)):
            for stl in stls:
                st = 4 * qc + stl
                sl = slice(stl * 128, (stl + 1) * 128)
                for oc in range(2):
                    ocs = slice(oc * 512, (oc + 1) * 512)
                    yp = ps_p.tile([128, 512], f32, tag="p512", name="p512")
                    for hpp in range(4):
                        nc.tensor.matmul(yp[:], OTn[hpp][qc][:, sl],
                                         wo[hpp][:, ocs],
                                         start=(hpp == 0), stop=(hpp == 3),
                                         skip_group_check=True)
                    ysb = work.tile([128, 512], f32, tag="ysb", name="ysb")
                    nc.vector.scalar_tensor_tensor(ysb[:], yp[:], 1.0,
                                                   bo_t[:, ocs], mult, add)
                    nc.sync.dma_start(y_d[st * 128 : (st + 1) * 128, ocs],
                                      ysb[:])

        for sc in range(4):
            # QKV projections for this seq-chunk. Q first: attention(qc>=1)
            # is gated on this chunk's Q (older K/V chunks are already done).
            for ct in range(4):
                cs = slice(ct * 128, (ct + 1) * 128)
                p = ps_p.tile([128, 512], f32, tag="p512", name="p512")
                for d in range(8):
                    nc.tensor.matmul(p[:], wq[d][:, cs], xt[d][sc][:],
                                     start=(d == 0), stop=(d == 7))
                nc.vector.tensor_scalar_add(QT[ct][sc][:], p[:], bq[ct][:])
            for ct in range(4):
                cs = slice(ct * 128, (ct + 1) * 128)
                p = ps_p.tile([128, 512], f32, tag="p512", name="p512")
                for d in range(8):
                    nc.tensor.matmul(p[:], wk[d][:, cs], xt[d][sc][:],
                                     start=(d == 0), stop=(d == 7))
                nc.vector.tensor_scalar_add(KT[ct][sc][:], p[:], bk[ct][:])
            for stl in range(4):
                st = 4 * sc + stl
                ts = slice(stl * 128, (stl + 1) * 128)
                p = ps_p.tile([128, 512], f32, tag="p512", name="p512")
                for d in range(8):
                    nc.tensor.matmul(p[:], xt[d][sc][:, ts], wv[d][:, :],
                                     start=(d == 0), stop=(d == 7))
                v3 = V[st][:].rearrange("p (h e) -> p h e", e=65)
                nc.vector.scalar_tensor_tensor(
                    v3[:, :, 0:64],
                    p[:].rearrange("p (h e) -> p h e", e=64),
                    1.0,
                    bv_t[:].rearrange("p (h e) -> p h e", e=64),
                    mult, add,
                )
                nc.vector.memset(v3[:, :, 64:65], 1.0)

            # attention for query chunk qc = sc, head pairs interleaved so the
            # even head's K=64 matmuls (rows 0-63) and the odd head's (rows
            # 64-127) run concurrently in the PE array.
            qc = sc
            nkt = 4 * (qc + 1)
            for hp in range(4):
                h0, h1 = 2 * hp, 2 * hp + 1
                av0 = ps_av.tile([65, 512], f32, tag="av", name="av0")
                av1 = ps_av.tile([65, 512], f32, tag="av", name="av1")

                # software pipeline: S(kt) -> exp(kt) on ACT while PE runs
                # S(kt+1); AV(kt) issues after S(kt+1) so PE never waits exp.
                pend = []  # (kt, pt tile) awaiting AV

                def emit_av(kt, pt, first, last):
                    delta = max(0, 128 * kt - 512 * qc)
                    nc.tensor.matmul(
                        av0[:, delta:512], V[kt][:, h0 * 65 : h0 * 65 + 65],
                        pt[:, delta:512],
                        start=first, stop=last, skip_group_check=True)
                    nc.tensor.matmul(
                        av1[:, delta:512], V[kt][:, h1 * 65 : h1 * 65 + 65],
                        pt[:, 512 + delta : 1024],
                        start=first, stop=last, skip_group_check=True)

                for kt in range(nkt):
                    # merged S^T tile: cols 0-511 head h0, 512-1023 head h1
                    sp = ps_s.tile([128, 1024], f32, tag="s2", name="sp")
                    kcol = slice((kt % 4) * 128, (kt % 4) * 128 + 128)
                    diag = kt >= 4 * qc
                    delta = max(0, 128 * kt - 512 * qc)
                    nc.tensor.matmul(
                        sp[:, delta:512], KT[hp][kt // 4][0:64, kcol],
                        QT[hp][qc][0:64, delta:512],
                        start=True, stop=True, skip_group_check=True)
                    nc.tensor.matmul(
                        sp[:, 512 + delta : 1024], KT[hp][kt // 4][64:128, kcol],
                        QT[hp][qc][64:128, delta:512],
                        start=True, stop=True, skip_group_check=True)
                    pt = ptp.tile([128, 1024], f16, tag="pt", name="pt")
                    sp3 = sp[:].rearrange("p (h e) -> p h e", e=512)
                    pt3 = pt[:].rearrange("p (h e) -> p h e", e=512)
                    nc.scalar.activation(pt3[:, :, delta:512],
                                         sp3[:, :, delta:512], Exp, scale=SCALE)
                    if diag:
                        # zero the partially-masked 128-wide sub-block via a
                        # 0/1 upper-triangular mask (both heads in one op)
                        nc.vector.tensor_mul(
                            pt3[:, :, delta : delta + 128],
                            pt3[:, :, delta : delta + 128],
                            msk_t[:].rearrange("p (h e) -> p h e", e=128),
                        )
                    pend.append((kt, pt))
                    if len(pend) > 1:
                        k0, p0 = pend.pop(0)
                        emit_av(k0, p0, k0 == 0, False)
                k0, p0 = pend.pop(0)
                emit_av(k0, p0, k0 == 0, True)
                # softmax normalization per head (row 64 of av = denom).
                # reciprocal_approx_fast misreads PSUM sources, so stage the
                # denominator row into SBUF first (on DVE: the strict ACT
                # FIFO would head-of-line-block the next head-pair's exps).
                den0 = work.tile([1, 512], f32, tag="den", name="den0")
                nc.vector.tensor_copy(den0[:], av0[64:65, :])
                r0 = work.tile([1, 512], f32, tag="r", name="r0")
                nc.vector.reciprocal_approx_fast(r0[:], den0[:])
                rb0 = work.tile([64, 512], f32, tag="rb", name="rb0")
                nc.gpsimd.partition_broadcast(rb0[:], r0[:], channels=64)
                nc.vector.tensor_mul(OTn[hp][qc][0:64, :], av0[0:64, :], rb0[:])
                den1 = work.tile([1, 512], f32, tag="den", name="den1")
                nc.vector.tensor_copy(den1[:], av1[64:65, :])
                r1 = work.tile([1, 512], f32, tag="r", name="r1")
                nc.vector.reciprocal_approx_fast(r1[:], den1[:])
                rb1 = work.tile([64, 512], f32, tag="rb", name="rb1")
                nc.gpsimd.partition_broadcast(rb1[:], r1[:], channels=64)
                nc.vector.tensor_mul(OTn[hp][qc][64:128, :], av1[0:64, :], rb1[:])

                # out-projections for chunks 0-2 are deferred into the
                # (ACT-bound) attention window of chunk 3, where the PE
                # otherwise starves; hold back two chunk-2 groups to fill
                # the final normalization-chain stall.
                if sc == 3 and hp < 3:
                    emit_outproj(hp, (0, 1, 2, 3) if hp < 2 else (0, 1, 2))
        emit_outproj(2, (3,))
        emit_outproj(3)

    nc.compile()
    return nc


def _host_inputs(x, w_qkv, b_qkv, w_o, b_o):
    """Per-core input dicts implementing the sharding + layout prep."""
    # 0/1 upper-triangular mask (keep col >= row), duplicated for both heads
    M = np.triu(np.ones((128, 128), np.float16))
    msk = np.concatenate([M, M], axis=1)

    in_maps = []
    for c in range(N_CORES):
        b = c // 2
        hs = (c % 2) * HPC
        cols = slice(hs * DH, (hs + HPC) * DH)
        in_maps.append({
            "xT": np.ascontiguousarray(x[b].T).astype(np.float16),
            "wq": w_qkv[:, cols].astype(np.float16),
            "wk": w_qkv[:, D:][:, cols].astype(np.float16),
            "wv": w_qkv[:, 2 * D:][:, cols].astype(np.float16),
            "wo": w_o[hs * DH : (hs + HPC) * DH, :].astype(np.float16),
            "bq": b_qkv[cols].reshape(CH, 1).astype(np.float32),
            "bk": b_qkv[D:][cols].reshape(CH, 1).astype(np.float32),
            "bvb": np.tile(b_qkv[2 * D:][cols].astype(np.float32), (128, 1)),
            "bob": np.tile(b_o.astype(np.float32), (128, 1)),
            "msk": msk,
        })
    return in_maps


def kernel(x, w_qkv, b_qkv, w_o, b_o):
    global _cached
    from concourse.bass_utils import run_bass_kernel_spmd

    x = np.asarray(x)
    w_qkv = np.asarray(w_qkv)
    b_qkv = np.asarray(b_qkv)
    w_o = np.asarray(w_o)
    b_o = np.asarray(b_o)

    if _cached is None:
        _cached = _build_program()
    nc = _cached

    in_maps = _host_inputs(x, w_qkv, b_qkv, w_o, b_o)
    res = run_bass_kernel_spmd(nc, in_maps, list(range(N_CORES)))

    out = np.empty((B, N, D), np.float32)
    for b in range(B):
        out[b] = res.results[2 * b]["y"] + res.results[2 * b + 1]["y"]
    return out


# revision 22
# speedup vs baseline: 1.2609x; 1.0035x over previous
"""Causal self-attention on 8 Trainium2 NeuronCores.

Sharding: core c handles batch b = c//2 and heads [(c%2)*8, (c%2)*8+8).
Each core computes the full QKV projection for its head slice, causal
flash-style attention, and the row-parallel w_o partial product. The two
partials per batch are summed on the host (no device collectives).

All PE matmuls run in fp16 (1 cycle/row) with fp32 PSUM accumulation.
Feature-major layouts throughout:
  x^T [D, N]        (host pre-transposed)
  Q^T, K^T [ch, N]  (from GEMM with W stationary, x^T moving)
  V [N, ch] + ones  (from GEMM with x^T stationary, W moving)
  S^T [k, q] = K^T_tile.T @ Q^T  -> exp -> P^T [k, q]
  O^T [ch, q] = (V|1).T @ P^T    (row 64 = softmax denominator)
  y = O^T_norm.T @ W_o           (accumulated over ch tiles)

Causal masking (triangular tightening): for a diagonal-straddling S^T
block with offset delta = 128*kt - 512*qc, columns j < delta are fully
masked so S/exp/AV are simply narrowed to cols [delta, 512). The
partially-masked 128-wide sub-block [delta, delta+128) is zeroed after
exp by one DVE multiply with a 0/1 upper-triangular mask.
"""

import numpy as np

B, N, D, H = 4, 2048, 1024, 16
DH = 64
N_CORES = 8
HPC = 8            # heads per core
CH = HPC * DH      # 512 channels per core
SCALE = 1.0 / 8.0  # 1/sqrt(DH)

_cached = None


def _build_program():
    from contextlib import ExitStack

    import concourse.tile as tile
    from concourse import bacc, mybir

    f16 = mybir.dt.float16
    f32 = mybir.dt.float32
    Exp = mybir.ActivationFunctionType.Exp
    mult = mybir.AluOpType.mult
    add = mybir.AluOpType.add

    nc = bacc.Bacc(
        "TRN2", target_bir_lowering=False, debug=False, num_devices=N_CORES
    )

    xT_d = nc.dram_tensor("xT", [D, N], f16, kind="ExternalInput").ap()
    wq_d = nc.dram_tensor("wq", [D, CH], f16, kind="ExternalInput").ap()
    wk_d = nc.dram_tensor("wk", [D, CH], f16, kind="ExternalInput").ap()
    wv_d = nc.dram_tensor("wv", [D, CH], f16, kind="ExternalInput").ap()
    wo_d = nc.dram_tensor("wo", [CH, D], f16, kind="ExternalInput").ap()
    bq_d = nc.dram_tensor("bq", [CH, 1], f32, kind="ExternalInput").ap()
    bk_d = nc.dram_tensor("bk", [CH, 1], f32, kind="ExternalInput").ap()
    bv_d = nc.dram_tensor("bvb", [128, CH], f32, kind="ExternalInput").ap()
    bo_d = nc.dram_tensor("bob", [128, D], f32, kind="ExternalInput").ap()
    msk_d = nc.dram_tensor("msk", [128, 256], f16, kind="ExternalInput").ap()
    y_d = nc.dram_tensor("y", [N, D], f16, kind="ExternalOutput").ap()

    with tile.TileContext(nc) as tc, ExitStack() as ctx:
        const = ctx.enter_context(tc.tile_pool(name="const", bufs=1))
        actp = ctx.enter_context(tc.tile_pool(name="actp", bufs=1))
        work = ctx.enter_context(tc.tile_pool(name="work", bufs=3))
        ptp = ctx.enter_context(tc.tile_pool(name="ptp", bufs=5))
        ps_s = ctx.enter_context(tc.tile_pool(name="ps_s", bufs=2, space="PSUM"))
        ps_av = ctx.enter_context(tc.tile_pool(name="ps_av", bufs=2, space="PSUM"))
        ps_p = ctx.enter_context(tc.tile_pool(name="ps_p", bufs=2, space="PSUM"))

        # ---- constants / weights into SBUF ----
        # K-weights + first seq-chunk of x first so the K^T GEMM starts ASAP.
        wq = [const.tile([128, CH], f16, tag=f"wq{i}", name=f"wq{i}") for i in range(8)]
        wk = [const.tile([128, CH], f16, tag=f"wk{i}", name=f"wk{i}") for i in range(8)]
        wv = [const.tile([128, CH], f16, tag=f"wv{i}", name=f"wv{i}") for i in range(8)]
        xt = [[const.tile([128, 512], f16, tag=f"xt{i}_{sc}", name=f"xt{i}_{sc}")
               for sc in range(4)] for i in range(8)]
        # Round-robin input DMAs across engine queues so the 2D
        # row-descriptor processing runs in parallel; first-needed first.
        engs = [nc.sync, nc.gpsimd]
        _ei = [0]

        def dma_in(dst, src):
            engs[_ei[0] % len(engs)].dma_start(dst, src)
            _ei[0] += 1

        for i in range(8):
            dma_in(wq[i][:], wq_d[i * 128 : (i + 1) * 128, :])
            dma_in(xt[i][0][:], xT_d[i * 128 : (i + 1) * 128, 0:512])
        bq = [const.tile([128, 1], f32, tag=f"bq{j}", name=f"bq{j}") for j in range(4)]
        bk = [const.tile([128, 1], f32, tag=f"bk{j}", name=f"bk{j}") for j in range(4)]
        for j in range(4):
            dma_in(bq[j][:], bq_d[j * 128 : (j + 1) * 128, :])
            dma_in(bk[j][:], bk_d[j * 128 : (j + 1) * 128, :])
        for i in range(8):
            dma_in(wk[i][:], wk_d[i * 128 : (i + 1) * 128, :])
        bv_t = const.tile([128, CH], f32, tag="bvb", name="bvb")
        dma_in(bv_t[:], bv_d[:])
        msk_t = const.tile([128, 256], f16, tag="msk", name="msk")
        dma_in(msk_t[:], msk_d[:])
        for i in range(8):
            dma_in(wv[i][:], wv_d[i * 128 : (i + 1) * 128, :])
        for sc in range(1, 4):
            for i in range(8):
                dma_in(xt[i][sc][:],
                       xT_d[i * 128 : (i + 1) * 128, sc * 512 : (sc + 1) * 512])
        wo = [const.tile([128, D], f16, tag=f"wo{j}", name=f"wo{j}") for j in range(4)]
        for j in range(4):
            dma_in(wo[j][:], wo_d[j * 128 : (j + 1) * 128, :])
        bo_t = const.tile([128, D], f32, tag="bob", name="bob")
        dma_in(bo_t[:], bo_d[:])

        # ---- persistent activations ----
        QT = [[actp.tile([128, 512], f16, tag=f"qt{ct}_{sc}", name=f"qt{ct}_{sc}") for sc in range(4)]
              for ct in range(4)]
        KT = [[actp.tile([128, 512], f16, tag=f"kt{ct}_{sc}", name=f"kt{ct}_{sc}") for sc in range(4)]
              for ct in range(4)]
        V = [actp.tile([128, 8 * 65], f16, tag=f"v{st}", name=f"v{st}") for st in range(16)]
        OTn = [[actp.tile([128, 512], f16, tag=f"otn{hp}_{qc}", name=f"otn{hp}_{qc}") for qc in range(4)]
               for hp in range(4)]

        # ---- fused pipeline: per seq-chunk sc emit QKV(sc), then the
        # PREVIOUS chunk's out-projection (fills the PE while this chunk's
        # softmax normalization chain drains), then attention qc=sc.
        def emit_outproj(qc, stls=(0, 1, 2, 3)):
            for stl in stls:
                st = 4 * qc + stl
                sl = slice(stl * 128, (stl + 1) * 128)
                for oc in range(2):
                    ocs = slice(oc * 512, (oc + 1) * 512)
                    yp = ps_p.tile([128, 512], f32, tag="p512", name="p512")
                    for hpp in range(4):
                        nc.tensor.matmul(yp[:], OTn[hpp][qc][:, sl],
                                         wo[hpp][:, ocs],
                                         start=(hpp == 0), stop=(hpp == 3),
                                         skip_group_check=True)
                    ysb = work.tile([128, 512], f16, tag="ysb", name="ysb")
                    nc.vector.scalar_tensor_tensor(ysb[:], yp[:], 1.0,
                                                   bo_t[:, ocs], mult, add)
                    nc.sync.dma_start(y_d[st * 128 : (st + 1) * 128, ocs],
                                      ysb[:])

        for sc in range(4):
            # QKV projections for this seq-chunk. Q first: attention(qc>=1)
            # is gated on this chunk's Q (older K/V chunks are already done).
            for ct in range(4):
                cs = slice(ct * 128, (ct + 1) * 128)
                p = ps_p.tile([128, 512], f32, tag="p512", name="p512")
                for d in range(8):
                    nc.tensor.matmul(p[:], wq[d][:, cs], xt[d][sc][:],
                                     start=(d == 0), stop=(d == 7))
                nc.vector.tensor_scalar_add(QT[ct][sc][:], p[:], bq[ct][:])
            for ct in range(4):
                cs = slice(ct * 128, (ct + 1) * 128)
                p = ps_p.tile([128, 512], f32, tag="p512", name="p512")
                for d in range(8):
                    nc.tensor.matmul(p[:], wk[d][:, cs], xt[d][sc][:],
                                     start=(d == 0), stop=(d == 7))
                nc.vector.tensor_scalar_add(KT[ct][sc][:], p[:], bk[ct][:])
            for stl in range(4):
                st = 4 * sc + stl
                ts = slice(stl * 128, (stl + 1) * 128)
                p = ps_p.tile([128, 512], f32, tag="p512", name="p512")
                for d in range(8):
                    nc.tensor.matmul(p[:], xt[d][sc][:, ts], wv[d][:, :],
                                     start=(d == 0), stop=(d == 7))
                v3 = V[st][:].rearrange("p (h e) -> p h e", e=65)
                nc.vector.scalar_tensor_tensor(
                    v3[:, :, 0:64],
                    p[:].rearrange("p (h e) -> p h e", e=64),
                    1.0,
                    bv_t[:].rearrange("p (h e) -> p h e", e=64),
                    mult, add,
                )
                nc.vector.memset(v3[:, :, 64:65], 1.0)

            # attention for query chunk qc = sc, head pairs interleaved so the
            # even head's K=64 matmuls (rows 0-63) and the odd head's (rows
            # 64-127) run concurrently in the PE array.
            qc = sc
            nkt = 4 * (qc + 1)
            for hp in range(4):
                h0, h1 = 2 * hp, 2 * hp + 1
                av0 = ps_av.tile([65, 512], f32, tag="av", name="av0")
                av1 = ps_av.tile([65, 512], f32, tag="av", name="av1")

                # software pipeline: S(kt) -> exp(kt) on ACT while PE runs
                # S(kt+1); AV(kt) issues after S(kt+1) so PE never waits exp.
                pend = []  # (kt, pt tile) awaiting AV

                def emit_av(kt, pt, first, last):
                    delta = max(0, 128 * kt - 512 * qc)
                    nc.tensor.matmul(
                        av0[:, delta:512], V[kt][:, h0 * 65 : h0 * 65 + 65],
                        pt[:, delta:512],
                        start=first, stop=last, skip_group_check=True)
                    nc.tensor.matmul(
                        av1[:, delta:512], V[kt][:, h1 * 65 : h1 * 65 + 65],
                        pt[:, 512 + delta : 1024],
                        start=first, stop=last, skip_group_check=True)

                for kt in range(nkt):
                    # merged S^T tile: cols 0-511 head h0, 512-1023 head h1
                    sp = ps_s.tile([128, 1024], f32, tag="s2", name="sp")
                    kcol = slice((kt % 4) * 128, (kt % 4) * 128 + 128)
                    diag = kt >= 4 * qc
                    delta = max(0, 128 * kt - 512 * qc)
                    nc.tensor.matmul(
                        sp[:, delta:512], KT[hp][kt // 4][0:64, kcol],
                        QT[hp][qc][0:64, delta:512],
                        start=True, stop=True, skip_group_check=True)
                    nc.tensor.matmul(
                        sp[:, 512 + delta : 1024], KT[hp][kt // 4][64:128, kcol],
                        QT[hp][qc][64:128, delta:512],
                        start=True, stop=True, skip_group_check=True)
                    pt = ptp.tile([128, 1024], f16, tag="pt", name="pt")
                    sp3 = sp[:].rearrange("p (h e) -> p h e", e=512)
                    pt3 = pt[:].rearrange("p (h e) -> p h e", e=512)
                    nc.scalar.activation(pt3[:, :, delta:512],
                                         sp3[:, :, delta:512], Exp, scale=SCALE)
                    if diag:
                        # zero the partially-masked 128-wide sub-block via a
                        # 0/1 upper-triangular mask (both heads in one op)
                        nc.vector.tensor_mul(
                            pt3[:, :, delta : delta + 128],
                            pt3[:, :, delta : delta + 128],
                            msk_t[:].rearrange("p (h e) -> p h e", e=128),
                        )
                    pend.append((kt, pt))
                    if len(pend) > 1:
                        k0, p0 = pend.pop(0)
                        emit_av(k0, p0, k0 == 0, False)
                k0, p0 = pend.pop(0)
                emit_av(k0, p0, k0 == 0, True)
                # softmax normalization per head (row 64 of av = denom).
                # reciprocal_approx_fast misreads PSUM sources, so stage the
                # denominator row into SBUF first (on DVE: the strict ACT
                # FIFO would head-of-line-block the next head-pair's exps).
                den0 = work.tile([1, 512], f32, tag="den", name="den0")
                nc.vector.tensor_copy(den0[:], av0[64:65, :])
                r0 = work.tile([1, 512], f32, tag="r", name="r0")
                nc.vector.reciprocal_approx_fast(r0[:], den0[:])
                rb0 = work.tile([64, 512], f32, tag="rb", name="rb0")
                nc.gpsimd.partition_broadcast(rb0[:], r0[:], channels=64)
                nc.vector.tensor_mul(OTn[hp][qc][0:64, :], av0[0:64, :], rb0[:])
                den1 = work.tile([1, 512], f32, tag="den", name="den1")
                nc.vector.tensor_copy(den1[:], av1[64:65, :])
                r1 = work.tile([1, 512], f32, tag="r", name="r1")
                nc.vector.reciprocal_approx_fast(r1[:], den1[:])
                rb1 = work.tile([64, 512], f32, tag="rb", name="rb1")
                nc.gpsimd.partition_broadcast(rb1[:], r1[:], channels=64)
                nc.vector.tensor_mul(OTn[hp][qc][64:128, :], av1[0:64, :], rb1[:])

                # out-projections for chunks 0-2 are deferred into the
                # (ACT-bound) attention window of chunk 3, where the PE
                # otherwise starves; hold back two chunk-2 groups to fill
                # the final normalization-chain stall.
                if sc == 3 and hp < 3:
                    emit_outproj(hp, (0, 1, 2, 3) if hp == 0 else (0, 1, 2))
        emit_outproj(1, (3,))
        emit_outproj(2, (3,))
        emit_outproj(3)

    nc.compile()
    return nc


def _host_inputs(x, w_qkv, b_qkv, w_o, b_o):
    """Per-core input dicts implementing the sharding + layout prep."""
    # 0/1 upper-triangular mask (keep col >= row), duplicated for both heads
    M = np.triu(np.ones((128, 128), np.float16))
    msk = np.concatenate([M, M], axis=1)

    in_maps = []
    for c in range(N_CORES):
        b = c // 2
        hs = (c % 2) * HPC
        cols = slice(hs * DH, (hs + HPC) * DH)
        in_maps.append({
            "xT": np.ascontiguousarray(x[b].T).astype(np.float16),
            "wq": w_qkv[:, cols].astype(np.float16),
            "wk": w_qkv[:, D:][:, cols].astype(np.float16),
            "wv": w_qkv[:, 2 * D:][:, cols].astype(np.float16),
            "wo": w_o[hs * DH : (hs + HPC) * DH, :].astype(np.float16),
            "bq": b_qkv[cols].reshape(CH, 1).astype(np.float32),
            "bk": b_qkv[D:][cols].reshape(CH, 1).astype(np.float32),
            "bvb": np.tile(b_qkv[2 * D:][cols].astype(np.float32), (128, 1)),
            "bob": np.tile(b_o.astype(np.float32), (128, 1)),
            "msk": msk,
        })
    return in_maps


def kernel(x, w_qkv, b_qkv, w_o, b_o):
    global _cached
    from concourse.bass_utils import run_bass_kernel_spmd

    x = np.asarray(x)
    w_qkv = np.asarray(w_qkv)
    b_qkv = np.asarray(b_qkv)
    w_o = np.asarray(w_o)
    b_o = np.asarray(b_o)

    if _cached is None:
        _cached = _build_program()
    nc = _cached

    in_maps = _host_inputs(x, w_qkv, b_qkv, w_o, b_o)
    res = run_bass_kernel_spmd(nc, in_maps, list(range(N_CORES)))

    out = np.empty((B, N, D), np.float32)
    for b in range(B):
        out[b] = (res.results[2 * b]["y"].astype(np.float32)
                  + res.results[2 * b + 1]["y"].astype(np.float32))
    return out
